# revision 38
# baseline (speedup 1.0000x reference)
"""Trainium2 Bass kernel for nn_Attention_88785563943675.

Single-head attention (reference reuses identical per-head weights; concat+WO
collapses to one [50,200] projection with WO_eff = sum of WO row blocks).

Per batch b:  Qp = q[b] WQ, Kp = k[b] WK, Vp = v[b] WV   [S, 50]
              A = softmax(Qp Kp^T / sqrt(50)),  O = A Vp,  Y = O WO_eff

Sharding: 8 cores = (batch 0..3) x (q-half 0..1); each core holds full k/v of
its batch and 2048 q rows.

Design (v2, vs the 123.7us exp-paced v1):
  - Host passes q/k/v pre-transposed and pre-cast: qT/kT/vT bf16 [200, s]
    (pure layout/dtype marshalling; all FLOPs stay on device), landing as
    [100, 2, s] d-chunk tiles in 512-col pieces.
  - 1024-wide exps: St accumulates into [128,1024] 2-bank PSUM tiles
    (2 bufs) and ScalarE runs TWO exps per k-block instead of four,
    amortizing the ~250-370ns fixed activation overhead (PSUM/SBUF access
    latency at 172/222 cycles, 1.2GHz ActE clock). Scalar/block ~2.2us.
  - PE warm-up spin (N_WARM identity transposes, no data deps) holds the
    Tensor engine busy from t~0 so it reaches the full 2.4GHz P-state
    (3us continuous-busy threshold; cold PE runs at 0.65-1.2GHz) before the
    first real projection, and bridges the ~8.5us DGE startup dead time.
  - Arrival-matched prologue at 512-col granularity on two DMA queues
    (SP: w,q0,q1,v0,k1-3; GpSimd SWDGE: k0,q2,q3,v1-3,rhs): St(0) half A
    needs only q0,q1,k0; q2/q3 projections are emitted between the two
    St(0) halves. First exp ~15us (v1: 24.8us).
  - Main loop over 32 k-blocks: 4x St [128,512] (shared KpT[kb] weights)
    into two [128,1024] tiles -> 2x ScalarE exp -> 4x AV into a persistent
    [128,2048] PSUM accumulator; pipelined emission (St(kb+1) before
    AV(kb)) + tile_wait_until bands keep the S^4 A^4 PE order = two
    weight-set switches per block. K/V projections + Vp crossbars ride
    arrival-matched per-block schedules through borrowed st-pool slots.
  - Epilogue interleaved per 512-chunk: AV(31) sub p -> OT evac (GpSimd) ->
    4x Yu fp32r matmuls (yu tiles borrow the st pool), reciprocal (DVE),
    1/l row scaling split Scalar/DVE, stores rotated over SP/Scalar/GpSimd
    queues so no single engine/queue serializes the tail.

Platform notes (axon TRN2): PE streams 512-col bf16 matmuls back-to-back at
~213-260ns once at full P-state; each weight-set change after a stream costs
~140ns (LD cannot prefetch across a pending stream); tile_position packing
never runs concurrently; fp8 fails tolerance; the tile scheduler's sim does
not model LDWEIGHTS cost or real DMA latency (real DGE: ~8.5us to first
byte, ~75GB/s/queue steady), so emission order + bands define the runtime
order. Any DMA issue on the Activation queue ahead of the exps head-of-line
blocks them (costs 5-30us), so inputs ride SP + the GpSimd software DGE and
only post-exp stores use the Activation queue.
enable-ldw-opt=true fails walrus codegen on this kernel (visitInstLdweights).
"""

import math

import numpy as np
import ml_dtypes

import concourse.bacc as bacc
import concourse.bass as bass
import concourse.mybir as mybir
import concourse.tile as tile
from concourse.bass_utils import run_bass_kernel_spmd

B = 4
S = 4096
D = 200
E = 50  # size per head
N_CORES = 8
SQ = S // 2  # q rows per core
SK = S  # k rows per core
SCALE = 1.0 / math.sqrt(E)

F32 = mybir.dt.float32
F32R = mybir.dt.float32r
BF16 = mybir.dt.bfloat16

ST_W = 512  # projection chunk width
DC = 100  # d-chunk size (two K=100 contraction chunks)
N_KB = SK // 128  # 32 k-blocks
N_QB = SQ // 128  # 16 q-blocks

N_WARM = 58  # PE warm-up transposes bridging DGE startup

# Arrival-matched schedules (block -> chunk). k chunk t is needed by St at
# kb=4t, v chunk t (plus its crossbar) by AV at kb=4t (emitted in block
# 4t+1). Loads: k4-7 ride SP, v4-7 ride GpSimd, queued behind the prologue
# pieces; both land >5 blocks before their projection slot.
K_PROJ = {2: 1, 4: 2, 6: 3, 10: 4, 14: 5, 18: 6, 22: 7}
V_PROJ = {3: 1, 5: 2, 7: 3, 11: 4, 15: 5, 19: 6, 23: 7}
K_LOAD = {3: 4, 6: 5, 9: 6, 12: 7}
V_LOAD = {4: 4, 7: 5, 10: 6, 13: 7}


def _emit(nc, tc, qT_ap, kT_ap, vT_ap, w_ap, rhs_ap, out_ap):
    import contextlib

    stack = contextlib.ExitStack()
    singles = stack.enter_context(tc.tile_pool(name="singles", bufs=1))

    # Warm-spin source: memset from DVE at t~0.3us, no const-load or DMA
    # dependency (make_identity's constants only land at ~9us).
    wsrc = singles.tile([128, 128], BF16)
    nc.vector.memset(wsrc, 1.0)

    # Weights [100, 3, 2, 50] bf16 from host (q/k/v x chunk0/1).
    w_bf = singles.tile([DC, 3, 2, E], BF16)
    nc.sync.dma_start(out=w_bf, in_=w_ap)

    # Raw transposed inputs, bf16, [100, 2 d-chunks, s]; 512-col pieces.
    xq = singles.tile([DC, 2, SQ], BF16)
    xk = singles.tile([DC, 2, SK], BF16)
    xv = singles.tile([DC, 2, SK], BF16)

    def load_piece(eng, x, x_ap, c0, c1):
        eng.dma_start(
            out=x[:, :, c0:c1],
            in_=x_ap[:, c0:c1].rearrange("(c p) s -> p c s", p=DC),
        )

    # Input streams, in per-queue consumption order. At most 8 pieces are
    # issued upfront on the GpSimd SWDGE queue: more recycles the 8-deep
    # DMA semaphore ring and the scheduler then gates OTHER queues (the SP
    # Vp-crossbars!) on late piece completions -- 10-15us AV stalls. The
    # rest (k4-7, v4-7) issue mid-loop via K_LOAD/V_LOAD. The Activation
    # queue carries NO input DMA.
    load_piece(nc.sync, xq, qT_ap, 0, 512)
    load_piece(nc.gpsimd, xk, kT_ap, 0, 512)
    load_piece(nc.gpsimd, xq, qT_ap, 512, 1024)
    load_piece(nc.gpsimd, xv, vT_ap, 0, 512)
    load_piece(nc.sync, xq, qT_ap, 1024, 1536)
    load_piece(nc.gpsimd, xq, qT_ap, 1536, 2048)
    for col in range(1, 4):
        load_piece(nc.sync, xk, kT_ap, col * ST_W, (col + 1) * ST_W)
        load_piece(nc.gpsimd, xv, vT_ap, col * ST_W, (col + 1) * ST_W)

    # Output-projection rhs [51, 256] f32 from host, behind v3 on GpSimd.
    # NOTE: the f32->f32r staging copy is emitted mid-loop (emit_st kb=16),
    # not here -- an early Vector-queue instruction waiting on this late
    # DMA would head-of-line block every projection evacuation.
    rhs_stage = singles.tile([E + 1, 256], F32)
    nc.gpsimd.dma_start(out=rhs_stage, in_=rhs_ap)
    rhs_aug = singles.tile([E + 1, 256], F32R)

    # Persistent projected tensors
    KpT = singles.tile([E, SK], BF16)  # [50, 4096]
    QpT = singles.tile([E, SQ], BF16)  # [50, 2048]
    VpT = singles.tile([64, SK], BF16)  # rows 0:50 = Vp^T, row 50 = ones
    # Engine partition starts must be 32-aligned: set rows 32:64 to 1.0 up
    # front; the projection evacuations then overwrite rows 32:50, leaving
    # row 50 (the denominator ones row) and unread rows 51:64 at 1.0.
    nc.vector.memset(VpT[32:64, :], 1.0)
    Vp = singles.tile([128, N_KB, 64], BF16)  # xbar of VpT; cols 0:51 used
    OT = singles.tile([E + 1, SQ], F32R)  # [51, 2048] O^T unnormalized + l

    # PE warm-up spin: no data deps (reads the memset wsrc), keeps the
    # Tensor engine continuously busy from ~0.5us through the ~9-12us DGE
    # startup so the P-state is at 2.4GHz (and the sequencer awake) when
    # the first projection input lands.
    with tc.tile_pool(name="warm_ps", bufs=1, space="PSUM") as warm_pool:
        warm = warm_pool.tile([128, 128], BF16, tag="warm")
        for _ in range(max(N_WARM, 1)):
            nc.tensor.transpose(out=warm, in_=wsrc, identity=wsrc)

    def project_group(pool, x, widx, dest, ts, with_xbar=False, tag="st"):
        """dest[:, t*512:(t+1)*512] for t in ts, grouped by weight chunk:
        all chunk-0 matmuls (one weight set), then all chunk-1."""
        pps = [
            pool.tile([E, ST_W], F32, tag=tag, name=f"pp{t}") for t in ts
        ]
        for pp, t in zip(pps, ts):
            nc.tensor.matmul(
                pp, lhsT=w_bf[:, widx, 0, :],
                rhs=x[:, 0, t * ST_W : (t + 1) * ST_W],
                start=True, stop=False,
            )
        for pp, t in zip(pps, ts):
            nc.tensor.matmul(
                pp, lhsT=w_bf[:, widx, 1, :],
                rhs=x[:, 1, t * ST_W : (t + 1) * ST_W],
                start=False, stop=True,
            )
        for pp, t in zip(pps, ts):
            nc.vector.tensor_copy(
                out=dest[0:E, t * ST_W : (t + 1) * ST_W], in_=pp
            )
        if with_xbar:
            for t in ts:
                # VpT slice -> Vp[:, 4t:4t+4, :]:
                # Vp[p, 4t+j, c] = VpT[c, t*512 + j*128 + p]
                # DMA transpose is HWDGE-only; it rides SP, which carries
                # no mid-loop input loads (all on the GpSimd SWDGE) so the
                # xbar never queues behind a 205KB k/v piece.
                nc.sync.dma_start_transpose(
                    out=Vp[:, 4 * t : 4 * (t + 1), :],
                    in_=VpT[:, t * ST_W : (t + 1) * ST_W],
                )

    # ---- Main loop: 32 k-blocks, full q width, pipelined emission -------
    # PSUM: st 2x[128,1024] (4 banks) + ot [128,2048] (4 banks). The
    # prologue q/k/v projections, the in-loop projections AND the epilogue
    # yu matmuls all borrow st-pool slots.
    with (
        tc.tile_pool(name="pt", bufs=3) as pt_pool,
        tc.tile_pool(name="st_ps", bufs=4, space="PSUM") as st_psum,
        tc.tile_pool(name="ot_ps", bufs=1, space="PSUM") as ot_psum,
        tc.tile_pool(name="fin", bufs=10) as fin_pool,
    ):
        ot = ot_psum.tile([128, SQ], F32, tag="ot")  # rows 0:51 used

        def st_sub(kb, sub, pt):
            st = st_psum.tile([128, 512], F32, tag="st")
            nc.tensor.matmul(
                st,
                lhsT=KpT[:, kb * 128 : (kb + 1) * 128],
                rhs=QpT[:, sub * 512 : (sub + 1) * 512],
                start=True, stop=True,
            )
            nc.scalar.activation(
                out=pt[:, sub * 512 : (sub + 1) * 512], in_=st,
                func=mybir.ActivationFunctionType.Exp, scale=SCALE,
            )

        def st_half(kb, h, pt):
            st_sub(kb, 2 * h, pt)
            st_sub(kb, 2 * h + 1, pt)

        def emit_st(kb):
            if kb in K_LOAD:
                t = K_LOAD[kb]
                load_piece(nc.sync, xk, kT_ap, t * ST_W, (t + 1) * ST_W)
            if kb in V_LOAD:
                t = V_LOAD[kb]
                load_piece(nc.gpsimd, xv, vT_ap, t * ST_W, (t + 1) * ST_W)
            if kb in K_PROJ:
                project_group(st_psum, xk, 1, KpT, [K_PROJ[kb]])
            if kb in V_PROJ:
                project_group(st_psum, xv, 2, VpT, [V_PROJ[kb]],
                              with_xbar=True)
            if kb == 16:
                nc.vector.tensor_copy(out=rhs_aug, in_=rhs_stage)
            pt = pt_pool.tile([128, SQ], BF16, tag="pt")
            st_half(kb, 0, pt)
            st_half(kb, 1, pt)
            return pt

        def emit_av(kb, pt):
            for sub in range(4):
                nc.tensor.matmul(
                    ot[0 : E + 1, sub * 512 : (sub + 1) * 512],
                    lhsT=Vp[:, kb, 0 : E + 1],
                    rhs=pt[:, sub * 512 : (sub + 1) * 512],
                    start=(kb == 0), stop=(kb == N_KB - 1),
                )

        # Scheduler bands (sim-only wait timestamps, no hw effect): dictate
        # the PE order S^4 A^4 per block so each block pays exactly two
        # weight-set switches; the greedy scheduler otherwise alternates
        # around exp readiness and pays ~6. B0 clears the warm-spin +
        # prologue sim time.
        B0 = 0.016
        BAND_MS = 0.01

        # ---- Block 0, interleaved with the arrival-matched prologue -----
        with tc.tile_wait_until(B0):
            project_group(st_psum, xq, 0, QpT, [0])
            project_group(st_psum, xq, 0, QpT, [1])
            project_group(st_psum, xk, 1, KpT, [0])
            # v0 proj + crossbar BEFORE the St stream: AV(0) needs the xbar,
            # and a later-emitted xbar gets scheduler-sequenced behind
            # mid-loop milestones on the SP queue (a 5-10us AV(0) stall).
            project_group(st_psum, xv, 2, VpT, [0], with_xbar=True)
            pt0 = pt_pool.tile([128, SQ], BF16, tag="pt")
            st_half(0, 0, pt0)
            project_group(st_psum, xq, 0, QpT, [2, 3])
            st_half(0, 1, pt0)
            prev_pt = pt0

        for kb in range(1, N_KB):
            with tc.tile_wait_until(B0 + BAND_MS * kb):
                cur_pt = emit_st(kb)
                emit_av(kb - 1, prev_pt)
                prev_pt = cur_pt

        # ---- Epilogue, interleaved per 512-chunk ------------------------
        # AV(31) sub p finalizes ot cols [p*512,(p+1)*512); evacuate that
        # chunk (GpSimd) and run its 4 Yu matmuls while later subs land.
        # Yu = [O_unnorm | l] @ rhs_aug at fp32r (N=256 -> full rate); rows
        # scaled by 1/l (DVE reciprocal + Scalar/DVE scaled copy).
        with tc.tile_wait_until(B0 + BAND_MS * N_KB):
            # All four AV(31) writes BEFORE any ot read: tile-granular WAR
            # tracking makes each ot write wait on any earlier-emitted ot
            # read, so interleaving write/read serializes at ~1.2us/sub.
            for p in range(4):
                nc.tensor.matmul(
                    ot[0 : E + 1, p * 512 : (p + 1) * 512],
                    lhsT=Vp[:, N_KB - 1, 0 : E + 1],
                    rhs=prev_pt[:, p * 512 : (p + 1) * 512],
                    start=False, stop=True,
                )
            # Evacuations: chunk 0 on DVE (free at that moment, unblocks
            # Yu(0) fastest), rest on ScalarE which is idle after the exps
            # (GpSimd cannot read PSUM).
            for p in range(4):
                if p == 0:
                    nc.vector.tensor_copy(
                        out=OT[:, p * 512 : (p + 1) * 512],
                        in_=ot[0 : E + 1, p * 512 : (p + 1) * 512],
                    )
                else:
                    nc.scalar.activation(
                        out=OT[:, p * 512 : (p + 1) * 512],
                        in_=ot[0 : E + 1, p * 512 : (p + 1) * 512],
                        func=mybir.ActivationFunctionType.Copy,
                    )
        with tc.tile_wait_until(B0 + BAND_MS * (N_KB + 1)):
            store_eng = [
                nc.sync, nc.scalar, nc.sync, nc.gpsimd,
                nc.sync, nc.sync, nc.sync, nc.scalar,
                nc.sync, nc.gpsimd, nc.sync, nc.scalar,
                nc.sync, nc.gpsimd, nc.sync, nc.sync,
            ]
            for qb in range(N_QB):
                yu = st_psum.tile([128, 256], F32, tag="st")
                nc.tensor.matmul(
                    yu,
                    lhsT=OT[:, qb * 128 : (qb + 1) * 128],
                    rhs=rhs_aug,
                    start=True, stop=True,
                )
                ot_out = fin_pool.tile([128, D], BF16, tag="fout")
                rec = fin_pool.tile([128, 1], F32, tag="rec")
                nc.vector.reciprocal(rec, yu[:, 200:201])
                if qb % 2 == 0:
                    nc.scalar.activation(
                        out=ot_out, in_=yu[:, 0:D],
                        func=mybir.ActivationFunctionType.Copy, scale=rec,
                    )
                else:
                    nc.vector.tensor_scalar_mul(ot_out, yu[:, 0:D], rec)
                store_eng[qb].dma_start(
                    out=out_ap[qb * 128 : (qb + 1) * 128, :], in_=ot_out
                )

    stack.close()


_NC_CACHE = None


def build_nc():
    global _NC_CACHE
    if _NC_CACHE is not None:
        return _NC_CACHE
    nc = bacc.Bacc(
        "TRN2", target_bir_lowering=False, debug=False, num_devices=N_CORES
    )
    qT_ap = nc.dram_tensor("qT", [D, SQ], BF16, kind="ExternalInput").ap()
    kT_ap = nc.dram_tensor("kT", [D, SK], BF16, kind="ExternalInput").ap()
    vT_ap = nc.dram_tensor("vT", [D, SK], BF16, kind="ExternalInput").ap()
    w_ap = nc.dram_tensor("w", [DC, 3, 2, E], BF16, kind="ExternalInput").ap()
    rhs_ap = nc.dram_tensor("rhs", [E + 1, 256], F32, kind="ExternalInput").ap()
    out_ap = nc.dram_tensor("out", [SQ, D], BF16, kind="ExternalOutput").ap()

    with tile.TileContext(nc) as tc:
        _emit(nc, tc, qT_ap, kT_ap, vT_ap, w_ap, rhs_ap, out_ap)
    nc.compile()
    _NC_CACHE = nc
    return nc


def make_in_maps(q, k, v, WQ, WK, WV, WO):
    q = np.asarray(q, np.float32)
    k = np.asarray(k, np.float32)
    v = np.asarray(v, np.float32)
    WQ = np.asarray(WQ, np.float32)
    WK = np.asarray(WK, np.float32)
    WV = np.asarray(WV, np.float32)
    WO = np.asarray(WO, np.float32)
    # All 4 heads share WQ/WK/WV, so concat+WO == O @ (sum of WO blocks)
    wo_eff = WO.reshape(4, E, D).sum(axis=0).astype(np.float32)

    # Weights in the device chunk layout [100, 3, 2, 50] bf16.
    w_stage = np.zeros((DC, 3, 2, E), np.float32)
    for i, W in enumerate((WQ, WK, WV)):
        w_stage[:, i, 0, :] = W[0:DC, :]
        w_stage[:, i, 1, :] = W[DC:D, :]
    w_dev = w_stage.astype(ml_dtypes.bfloat16)

    # Output-projection rhs [51, 256]: rows 0:50 cols 0:200 = WO_eff,
    # row 50 col 200 = 1.0 (passes the softmax denominator l through).
    rhs = np.zeros((E + 1, 256), np.float32)
    rhs[0:E, 0:D] = wo_eff
    rhs[E, 200] = 1.0

    in_maps = []
    for c in range(N_CORES):
        b, h = c // 2, c % 2
        qT = np.ascontiguousarray(
            q[b, h * SQ : (h + 1) * SQ, :].T.astype(ml_dtypes.bfloat16)
        )
        kT = np.ascontiguousarray(k[b].T.astype(ml_dtypes.bfloat16))
        vT = np.ascontiguousarray(v[b].T.astype(ml_dtypes.bfloat16))
        in_maps.append({"qT": qT, "kT": kT, "vT": vT, "w": w_dev, "rhs": rhs})
    return in_maps


def assemble(results):
    out = np.empty((B, S, D), np.float32)
    for c in range(N_CORES):
        b, h = c // 2, c % 2
        out[b, h * SQ : (h + 1) * SQ, :] = np.asarray(
            results[c]["out"], dtype=np.float32
        )
    return out


def kernel(q, k, v, WQ, WK, WV, WO):
    nc = build_nc()
    in_maps = make_in_maps(q, k, v, WQ, WK, WV, WO)
    res = run_bass_kernel_spmd(nc, in_maps, core_ids=list(range(N_CORES)))
    return assemble(res.results)


if __name__ == "__main__":
    # quick self-run with random data
    rng = np.random.default_rng(0)
    q = rng.standard_normal((B, S, D)).astype(np.float32)
    k = rng.standard_normal((B, S, D)).astype(np.float32)
    v = rng.standard_normal((B, S, D)).astype(np.float32)
    WQ = rng.standard_normal((D, E)).astype(np.float32) * 0.08
    WK = rng.standard_normal((D, E)).astype(np.float32) * 0.08
    WV = rng.standard_normal((D, E)).astype(np.float32) * 0.08
    WO = rng.standard_normal((4 * E, D)).astype(np.float32) * 0.08
    out = kernel(q, k, v, WQ, WK, WV, WO)
    print("out", out.shape, out.dtype, np.abs(out).mean())


# revision 40
# speedup vs baseline: 1.0403x; 1.0403x over previous
"""Trainium2 Bass kernel for nn_Attention_88785563943675.

Single-head attention (reference reuses identical per-head weights; concat+WO
collapses to one [50,200] projection with WO_eff = sum of WO row blocks).

Per batch b:  Qp = q[b] WQ, Kp = k[b] WK, Vp = v[b] WV   [S, 50]
              A = softmax(Qp Kp^T / sqrt(50)),  O = A Vp,  Y = O WO_eff

Sharding: 8 cores = (batch 0..3) x (q-half 0..1); each core holds full k/v of
its batch and 2048 q rows.

Design (v2, vs the 123.7us exp-paced v1):
  - Host passes q/k/v pre-transposed and pre-cast: qT/kT/vT bf16 [200, s]
    (pure layout/dtype marshalling; all FLOPs stay on device), landing as
    [100, 2, s] d-chunk tiles in 512-col pieces.
  - 1024-wide exps: St accumulates into [128,1024] 2-bank PSUM tiles
    (2 bufs) and ScalarE runs TWO exps per k-block instead of four,
    amortizing the ~250-370ns fixed activation overhead (PSUM/SBUF access
    latency at 172/222 cycles, 1.2GHz ActE clock). Scalar/block ~2.2us.
  - PE warm-up spin (N_WARM identity transposes, no data deps) holds the
    Tensor engine busy from t~0 so it reaches the full 2.4GHz P-state
    (3us continuous-busy threshold; cold PE runs at 0.65-1.2GHz) before the
    first real projection, and bridges the ~8.5us DGE startup dead time.
  - Arrival-matched prologue at 512-col granularity on two DMA queues
    (SP: w,q0,q1,v0,k1-3; GpSimd SWDGE: k0,q2,q3,v1-3,rhs): St(0) half A
    needs only q0,q1,k0; q2/q3 projections are emitted between the two
    St(0) halves. First exp ~15us (v1: 24.8us).
  - Main loop over 32 k-blocks: 4x St [128,512] (shared KpT[kb] weights)
    into two [128,1024] tiles -> 2x ScalarE exp -> 4x AV into a persistent
    [128,2048] PSUM accumulator; pipelined emission (St(kb+1) before
    AV(kb)) + tile_wait_until bands keep the S^4 A^4 PE order = two
    weight-set switches per block. K/V projections + Vp crossbars ride
    arrival-matched per-block schedules through borrowed st-pool slots.
  - Epilogue interleaved per 512-chunk: AV(31) sub p -> OT evac (GpSimd) ->
    4x Yu fp32r matmuls (yu tiles borrow the st pool), reciprocal (DVE),
    1/l row scaling split Scalar/DVE, stores rotated over SP/Scalar/GpSimd
    queues so no single engine/queue serializes the tail.

Platform notes (axon TRN2): PE streams 512-col bf16 matmuls back-to-back at
~213-260ns once at full P-state; each weight-set change after a stream costs
~140ns (LD cannot prefetch across a pending stream); tile_position packing
never runs concurrently; fp8 fails tolerance; the tile scheduler's sim does
not model LDWEIGHTS cost or real DMA latency (real DGE: ~8.5us to first
byte, ~75GB/s/queue steady), so emission order + bands define the runtime
order. Any DMA issue on the Activation queue ahead of the exps head-of-line
blocks them (costs 5-30us), so inputs ride SP + the GpSimd software DGE and
only post-exp stores use the Activation queue.
enable-ldw-opt=true fails walrus codegen on this kernel (visitInstLdweights).
"""

import math

import numpy as np
import ml_dtypes

import concourse.bacc as bacc
import concourse.bass as bass
import concourse.mybir as mybir
import concourse.tile as tile
from concourse.bass_utils import run_bass_kernel_spmd

B = 4
S = 4096
D = 200
E = 50  # size per head
N_CORES = 8
SQ = S // 2  # q rows per core
SK = S  # k rows per core
SCALE = 1.0 / math.sqrt(E)

F32 = mybir.dt.float32
F32R = mybir.dt.float32r
BF16 = mybir.dt.bfloat16

ST_W = 512  # projection chunk width
DC = 100  # d-chunk size (two K=100 contraction chunks)
N_KB = SK // 128  # 32 k-blocks
N_QB = SQ // 128  # 16 q-blocks

N_WARM = 58  # PE warm-up transposes bridging DGE startup

# Arrival-matched schedules (block -> chunk). k chunk t is needed by St at
# kb=4t, v chunk t (plus its crossbar) by AV at kb=4t (emitted in block
# 4t+1). Loads: k4-7 ride SP, v4-7 ride GpSimd, queued behind the prologue
# pieces; both land >5 blocks before their projection slot.
K_PROJ = {2: 1, 4: 2, 6: 3, 10: 4, 14: 5, 18: 6, 22: 7}
V_PROJ = {3: 1, 5: 2, 7: 3, 11: 4, 15: 5, 19: 6, 23: 7}
K_LOAD = {3: 4, 6: 5, 9: 6, 12: 7}
V_LOAD = {4: 4, 7: 5, 10: 6, 13: 7}


def _emit(nc, tc, qT_ap, kT_ap, vT_ap, w_ap, rhs_ap, out_ap):
    import contextlib

    stack = contextlib.ExitStack()
    singles = stack.enter_context(tc.tile_pool(name="singles", bufs=1))

    # Warm-spin source: memset from DVE at t~0.3us, no const-load or DMA
    # dependency (make_identity's constants only land at ~9us).
    wsrc = singles.tile([128, 128], BF16)
    nc.vector.memset(wsrc, 1.0)

    # Weights [100, 3, 2, 50] bf16 from host (q/k/v x chunk0/1).
    w_bf = singles.tile([DC, 3, 2, E], BF16)
    nc.sync.dma_start(out=w_bf, in_=w_ap)

    # Raw transposed inputs, bf16, [100, 2 d-chunks, s]; 512-col pieces.
    xq = singles.tile([DC, 2, SQ], BF16)
    xk = singles.tile([DC, 2, SK], BF16)
    xv = singles.tile([DC, 2, SK], BF16)

    def load_piece(eng, x, x_ap, c0, c1):
        eng.dma_start(
            out=x[:, :, c0:c1],
            in_=x_ap[:, c0:c1].rearrange("(c p) s -> p c s", p=DC),
        )

    # Input streams, in per-queue consumption order. At most 8 pieces are
    # issued upfront on the GpSimd SWDGE queue: more recycles the 8-deep
    # DMA semaphore ring and the scheduler then gates OTHER queues (the SP
    # Vp-crossbars!) on late piece completions -- 10-15us AV stalls. The
    # rest (k4-7, v4-7) issue mid-loop via K_LOAD/V_LOAD. The Activation
    # queue carries NO input DMA.
    load_piece(nc.sync, xq, qT_ap, 0, 512)
    load_piece(nc.gpsimd, xk, kT_ap, 0, 512)
    load_piece(nc.sync, xq, qT_ap, 512, 1024)
    load_piece(nc.gpsimd, xv, vT_ap, 0, 512)
    load_piece(nc.sync, xq, qT_ap, 1024, 1536)
    load_piece(nc.gpsimd, xq, qT_ap, 1536, 2048)
    for col in range(1, 4):
        load_piece(nc.sync, xk, kT_ap, col * ST_W, (col + 1) * ST_W)
        load_piece(nc.gpsimd, xv, vT_ap, col * ST_W, (col + 1) * ST_W)

    # Output-projection rhs [51, 256] f32 from host, behind v3 on GpSimd.
    # NOTE: the f32->f32r staging copy is emitted mid-loop (emit_st kb=16),
    # not here -- an early Vector-queue instruction waiting on this late
    # DMA would head-of-line block every projection evacuation.
    rhs_stage = singles.tile([E + 1, 256], F32)
    nc.gpsimd.dma_start(out=rhs_stage, in_=rhs_ap)
    rhs_aug = singles.tile([E + 1, 256], F32R)

    # Persistent projected tensors
    KpT = singles.tile([E, SK], BF16)  # [50, 4096]
    QpT = singles.tile([E, SQ], BF16)  # [50, 2048]
    VpT = singles.tile([64, SK], BF16)  # rows 0:50 = Vp^T, row 50 = ones
    # Engine partition starts must be 32-aligned: set rows 32:64 to 1.0 up
    # front; the projection evacuations then overwrite rows 32:50, leaving
    # row 50 (the denominator ones row) and unread rows 51:64 at 1.0.
    nc.vector.memset(VpT[32:64, :], 1.0)
    Vp = singles.tile([128, N_KB, 64], BF16)  # xbar of VpT; cols 0:51 used
    OT = singles.tile([E + 1, SQ], F32R)  # [51, 2048] O^T unnormalized + l

    # PE warm-up spin: no data deps (reads the memset wsrc), keeps the
    # Tensor engine continuously busy from ~0.5us through the ~9-12us DGE
    # startup so the P-state is at 2.4GHz (and the sequencer awake) when
    # the first projection input lands.
    with tc.tile_pool(name="warm_ps", bufs=1, space="PSUM") as warm_pool:
        warm = warm_pool.tile([128, 128], BF16, tag="warm")
        for _ in range(max(N_WARM, 1)):
            nc.tensor.transpose(out=warm, in_=wsrc, identity=wsrc)

    def project_group(pool, x, widx, dest, ts, with_xbar=False, tag="st"):
        """dest[:, t*512:(t+1)*512] for t in ts, grouped by weight chunk:
        all chunk-0 matmuls (one weight set), then all chunk-1."""
        pps = [
            pool.tile([E, ST_W], F32, tag=tag, name=f"pp{t}") for t in ts
        ]
        for pp, t in zip(pps, ts):
            nc.tensor.matmul(
                pp, lhsT=w_bf[:, widx, 0, :],
                rhs=x[:, 0, t * ST_W : (t + 1) * ST_W],
                start=True, stop=False,
            )
        for pp, t in zip(pps, ts):
            nc.tensor.matmul(
                pp, lhsT=w_bf[:, widx, 1, :],
                rhs=x[:, 1, t * ST_W : (t + 1) * ST_W],
                start=False, stop=True,
            )
        for pp, t in zip(pps, ts):
            nc.vector.tensor_copy(
                out=dest[0:E, t * ST_W : (t + 1) * ST_W], in_=pp
            )
        if with_xbar:
            for t in ts:
                # VpT slice -> Vp[:, 4t:4t+4, :]:
                # Vp[p, 4t+j, c] = VpT[c, t*512 + j*128 + p]
                # DMA transpose is HWDGE-only; it rides SP, which carries
                # no mid-loop input loads (all on the GpSimd SWDGE) so the
                # xbar never queues behind a 205KB k/v piece.
                nc.sync.dma_start_transpose(
                    out=Vp[:, 4 * t : 4 * (t + 1), :],
                    in_=VpT[:, t * ST_W : (t + 1) * ST_W],
                )

    # ---- Main loop: 32 k-blocks, full q width, pipelined emission -------
    # PSUM: st 2x[128,1024] (4 banks) + ot [128,2048] (4 banks). The
    # prologue q/k/v projections, the in-loop projections AND the epilogue
    # yu matmuls all borrow st-pool slots.
    with (
        tc.tile_pool(name="pt", bufs=3) as pt_pool,
        tc.tile_pool(name="st_ps", bufs=4, space="PSUM") as st_psum,
        tc.tile_pool(name="ot_ps", bufs=1, space="PSUM") as ot_psum,
        tc.tile_pool(name="fin", bufs=10) as fin_pool,
    ):
        ot = ot_psum.tile([128, SQ], F32, tag="ot")  # rows 0:51 used

        def st_sub(kb, sub, pt):
            st = st_psum.tile([128, 512], F32, tag="st")
            nc.tensor.matmul(
                st,
                lhsT=KpT[:, kb * 128 : (kb + 1) * 128],
                rhs=QpT[:, sub * 512 : (sub + 1) * 512],
                start=True, stop=True,
            )
            nc.scalar.activation(
                out=pt[:, sub * 512 : (sub + 1) * 512], in_=st,
                func=mybir.ActivationFunctionType.Exp, scale=SCALE,
            )

        def st_half(kb, h, pt):
            st_sub(kb, 2 * h, pt)
            st_sub(kb, 2 * h + 1, pt)

        def emit_st(kb):
            if kb in K_LOAD:
                t = K_LOAD[kb]
                load_piece(nc.sync, xk, kT_ap, t * ST_W, (t + 1) * ST_W)
            if kb in V_LOAD:
                t = V_LOAD[kb]
                load_piece(nc.gpsimd, xv, vT_ap, t * ST_W, (t + 1) * ST_W)
            if kb in K_PROJ:
                project_group(st_psum, xk, 1, KpT, [K_PROJ[kb]])
            if kb in V_PROJ:
                project_group(st_psum, xv, 2, VpT, [V_PROJ[kb]],
                              with_xbar=True)
            if kb == 16:
                nc.vector.tensor_copy(out=rhs_aug, in_=rhs_stage)
            pt = pt_pool.tile([128, SQ], BF16, tag="pt")
            st_half(kb, 0, pt)
            st_half(kb, 1, pt)
            return pt

        def emit_av(kb, pt):
            for sub in range(4):
                nc.tensor.matmul(
                    ot[0 : E + 1, sub * 512 : (sub + 1) * 512],
                    lhsT=Vp[:, kb, 0 : E + 1],
                    rhs=pt[:, sub * 512 : (sub + 1) * 512],
                    start=(kb == 0), stop=(kb == N_KB - 1),
                )

        # Scheduler bands (sim-only wait timestamps, no hw effect): dictate
        # the PE order S^4 A^4 per block so each block pays exactly two
        # weight-set switches; the greedy scheduler otherwise alternates
        # around exp readiness and pays ~6. B0 clears the warm-spin +
        # prologue sim time.
        B0 = 0.016
        BAND_MS = 0.01

        # ---- Block 0, interleaved with the arrival-matched prologue -----
        with tc.tile_wait_until(B0):
            project_group(st_psum, xq, 0, QpT, [0])
            project_group(st_psum, xq, 0, QpT, [1])
            project_group(st_psum, xk, 1, KpT, [0])
            # v0 proj + crossbar BEFORE the St stream: AV(0) needs the xbar,
            # and a later-emitted xbar gets scheduler-sequenced behind
            # mid-loop milestones on the SP queue (a 5-10us AV(0) stall).
            project_group(st_psum, xv, 2, VpT, [0], with_xbar=True)
            pt0 = pt_pool.tile([128, SQ], BF16, tag="pt")
            st_half(0, 0, pt0)
            project_group(st_psum, xq, 0, QpT, [2, 3])
            st_half(0, 1, pt0)
            prev_pt = pt0

        for kb in range(1, N_KB):
            with tc.tile_wait_until(B0 + BAND_MS * kb):
                cur_pt = emit_st(kb)
                emit_av(kb - 1, prev_pt)
                prev_pt = cur_pt

        # ---- Epilogue, interleaved per 512-chunk ------------------------
        # AV(31) sub p finalizes ot cols [p*512,(p+1)*512); evacuate that
        # chunk (GpSimd) and run its 4 Yu matmuls while later subs land.
        # Yu = [O_unnorm | l] @ rhs_aug at fp32r (N=256 -> full rate); rows
        # scaled by 1/l (DVE reciprocal + Scalar/DVE scaled copy).
        with tc.tile_wait_until(B0 + BAND_MS * N_KB):
            # All four AV(31) writes BEFORE any ot read: tile-granular WAR
            # tracking makes each ot write wait on any earlier-emitted ot
            # read, so interleaving write/read serializes at ~1.2us/sub.
            for p in range(4):
                nc.tensor.matmul(
                    ot[0 : E + 1, p * 512 : (p + 1) * 512],
                    lhsT=Vp[:, N_KB - 1, 0 : E + 1],
                    rhs=prev_pt[:, p * 512 : (p + 1) * 512],
                    start=False, stop=True,
                )
            # Evacuations: chunk 0 on DVE (free at that moment, unblocks
            # Yu(0) fastest), rest on ScalarE which is idle after the exps
            # (GpSimd cannot read PSUM).
            for p in range(4):
                if p == 0:
                    nc.vector.tensor_copy(
                        out=OT[:, p * 512 : (p + 1) * 512],
                        in_=ot[0 : E + 1, p * 512 : (p + 1) * 512],
                    )
                else:
                    nc.scalar.activation(
                        out=OT[:, p * 512 : (p + 1) * 512],
                        in_=ot[0 : E + 1, p * 512 : (p + 1) * 512],
                        func=mybir.ActivationFunctionType.Copy,
                    )
        with tc.tile_wait_until(B0 + BAND_MS * (N_KB + 1)):
            # Stores ride the two HWDGE queues only: a GpSimd SWDGE store
            # pays ~1us of software descriptor generation right at the end.
            store_eng = [
                nc.sync, nc.scalar, nc.sync, nc.scalar,
                nc.sync, nc.scalar, nc.sync, nc.scalar,
                nc.sync, nc.scalar, nc.sync, nc.scalar,
                nc.sync, nc.scalar, nc.sync, nc.sync,
            ]
            for qb in range(N_QB):
                yu = st_psum.tile([128, 256], F32, tag="st")
                nc.tensor.matmul(
                    yu,
                    lhsT=OT[:, qb * 128 : (qb + 1) * 128],
                    rhs=rhs_aug,
                    start=True, stop=True,
                )
                ot_out = fin_pool.tile([128, D], BF16, tag="fout")
                rec = fin_pool.tile([128, 1], F32, tag="rec")
                nc.vector.reciprocal(rec, yu[:, 200:201])
                if qb % 2 == 0:
                    nc.scalar.activation(
                        out=ot_out, in_=yu[:, 0:D],
                        func=mybir.ActivationFunctionType.Copy, scale=rec,
                    )
                else:
                    nc.vector.tensor_scalar_mul(ot_out, yu[:, 0:D], rec)
                store_eng[qb].dma_start(
                    out=out_ap[qb * 128 : (qb + 1) * 128, :], in_=ot_out
                )

    stack.close()


_NC_CACHE = None


def build_nc():
    global _NC_CACHE
    if _NC_CACHE is not None:
        return _NC_CACHE
    nc = bacc.Bacc(
        "TRN2", target_bir_lowering=False, debug=False, num_devices=N_CORES
    )
    qT_ap = nc.dram_tensor("qT", [D, SQ], BF16, kind="ExternalInput").ap()
    kT_ap = nc.dram_tensor("kT", [D, SK], BF16, kind="ExternalInput").ap()
    vT_ap = nc.dram_tensor("vT", [D, SK], BF16, kind="ExternalInput").ap()
    w_ap = nc.dram_tensor("w", [DC, 3, 2, E], BF16, kind="ExternalInput").ap()
    rhs_ap = nc.dram_tensor("rhs", [E + 1, 256], F32, kind="ExternalInput").ap()
    out_ap = nc.dram_tensor("out", [SQ, D], BF16, kind="ExternalOutput").ap()

    with tile.TileContext(nc) as tc:
        _emit(nc, tc, qT_ap, kT_ap, vT_ap, w_ap, rhs_ap, out_ap)
    nc.compile()
    _NC_CACHE = nc
    return nc


def make_in_maps(q, k, v, WQ, WK, WV, WO):
    q = np.asarray(q, np.float32)
    k = np.asarray(k, np.float32)
    v = np.asarray(v, np.float32)
    WQ = np.asarray(WQ, np.float32)
    WK = np.asarray(WK, np.float32)
    WV = np.asarray(WV, np.float32)
    WO = np.asarray(WO, np.float32)
    # All 4 heads share WQ/WK/WV, so concat+WO == O @ (sum of WO blocks)
    wo_eff = WO.reshape(4, E, D).sum(axis=0).astype(np.float32)

    # Weights in the device chunk layout [100, 3, 2, 50] bf16.
    w_stage = np.zeros((DC, 3, 2, E), np.float32)
    for i, W in enumerate((WQ, WK, WV)):
        w_stage[:, i, 0, :] = W[0:DC, :]
        w_stage[:, i, 1, :] = W[DC:D, :]
    w_dev = w_stage.astype(ml_dtypes.bfloat16)

    # Output-projection rhs [51, 256]: rows 0:50 cols 0:200 = WO_eff,
    # row 50 col 200 = 1.0 (passes the softmax denominator l through).
    rhs = np.zeros((E + 1, 256), np.float32)
    rhs[0:E, 0:D] = wo_eff
    rhs[E, 200] = 1.0

    in_maps = []
    for c in range(N_CORES):
        b, h = c // 2, c % 2
        qT = np.ascontiguousarray(
            q[b, h * SQ : (h + 1) * SQ, :].T.astype(ml_dtypes.bfloat16)
        )
        kT = np.ascontiguousarray(k[b].T.astype(ml_dtypes.bfloat16))
        vT = np.ascontiguousarray(v[b].T.astype(ml_dtypes.bfloat16))
        in_maps.append({"qT": qT, "kT": kT, "vT": vT, "w": w_dev, "rhs": rhs})
    return in_maps


def assemble(results):
    out = np.empty((B, S, D), np.float32)
    for c in range(N_CORES):
        b, h = c // 2, c % 2
        out[b, h * SQ : (h + 1) * SQ, :] = np.asarray(
            results[c]["out"], dtype=np.float32
        )
    return out


def kernel(q, k, v, WQ, WK, WV, WO):
    nc = build_nc()
    in_maps = make_in_maps(q, k, v, WQ, WK, WV, WO)
    res = run_bass_kernel_spmd(nc, in_maps, core_ids=list(range(N_CORES)))
    return assemble(res.results)


if __name__ == "__main__":
    # quick self-run with random data
    rng = np.random.default_rng(0)
    q = rng.standard_normal((B, S, D)).astype(np.float32)
    k = rng.standard_normal((B, S, D)).astype(np.float32)
    v = rng.standard_normal((B, S, D)).astype(np.float32)
    WQ = rng.standard_normal((D, E)).astype(np.float32) * 0.08
    WK = rng.standard_normal((D, E)).astype(np.float32) * 0.08
    WV = rng.standard_normal((D, E)).astype(np.float32) * 0.08
    WO = rng.standard_normal((4 * E, D)).astype(np.float32) * 0.08
    out = kernel(q, k, v, WQ, WK, WV, WO)
    print("out", out.shape, out.dtype, np.abs(out).mean())
